# revision 9
# baseline (speedup 1.0000x reference)
"""Trainium2 Bass kernel for KeypointAlignmentLossL2 — split-path version.

Data-parallel over batch (1 NeuronCore per batch element). The dot product
needs f1[kp] and f2[kp] on the same partition, so both images share one
keypoint ordering: kps are sorted by their img2 pixel-row index.

  - img2: full pixel-major feature map bulk-loaded to SBUF (fp8, 3.1 MB,
    HWDGE — zero Q7 time). Sorted chunks of 128 kps touch only a narrow
    window of 128-pixel-row tiles, so gather+bilinear-lerp fuses into a few
    accumulating one-hot matmuls per chunk (sparse weight mats built on
    host, ~0.9 MB).
  - img1: dma_gather of 2-row pair spans per kp (sorted order; gather does
    not care about locality). Only image 1 pays Q7 descriptor-gen time
    (~17 us). A dummy 16-idx gather issued first forces the ~6 us Q7
    gather-ucode IRAM load to overlap the input DMAs.
  - lerp img1: diagonal-matrix matmuls; squares on ScalarE (PSUM read,
    accum_out); dot via scalar_tensor_tensor accum_out fusion on VectorE
    (tensor_tensor_reduce mis-executes on HW).
  Host finishes: masked mean of l2 distances across cores.
"""
import numpy as np
import ml_dtypes

B, C, H, W, N = 8, 768, 64, 64, 1024
HW_ = H * W
NCHUNK = N // 128   # 8 chunks of 128 keypoints
NTILE = HW_ // 128  # 32 map tiles of 128 pixel-rows
CH = C // 2         # 384 = one PSUM bank of f32
NG = 4              # img1 gather calls (512 idxs = 2 chunks each)

F8 = ml_dtypes.float8_e4m3

_CACHE = {}


def _build_nc(windows):
    """windows: tuple of (t_lo, span) per chunk for the img2 one-hot path."""
    from contextlib import ExitStack
    import concourse.bass as bass
    import concourse.tile as tile
    import concourse.mybir as mybir
    from concourse import bacc

    f32 = mybir.dt.float32
    f8 = mybir.dt.float8e4
    i16 = mybir.dt.int16

    sums = sum(s for _, s in windows)
    offs = np.cumsum([0] + [s for _, s in windows])[:-1]

    nc = bacc.Bacc("TRN2", target_bir_lowering=False, debug=False, num_devices=8)

    featT1 = nc.dram_tensor("featT1", [HW_, C], f8, kind="ExternalInput")
    featI2 = nc.dram_tensor("featI2", [128, NTILE * C], f8, kind="ExternalInput")
    idx1 = nc.dram_tensor("idx1", [128, 2 * N // 16], i16, kind="ExternalInput")
    wd1 = nc.dram_tensor("wd1", [128, NCHUNK * 4, 128], f8, kind="ExternalInput")
    w2m = nc.dram_tensor("w2m", [128, sums, 128], f8, kind="ExternalInput")
    out_n1 = nc.dram_tensor("out_n1", [128, 2 * NCHUNK], f32, kind="ExternalOutput")
    out_n2 = nc.dram_tensor("out_n2", [128, 2 * NCHUNK], f32, kind="ExternalOutput")
    out_dot = nc.dram_tensor("out_dot", [128, 2 * NCHUNK], f32, kind="ExternalOutput")

    MULT = mybir.AluOpType.mult

    fap1 = featT1[:]
    fap1.ap[0] = [C, HW_ - 1]
    fap1.ap[1] = [1, 2 * C]

    with tile.TileContext(nc) as tc, ExitStack() as ctx:
        const_pool = ctx.enter_context(tc.tile_pool(name="const", bufs=1))
        gpool = ctx.enter_context(tc.tile_pool(name="g", bufs=3))
        dpool = ctx.enter_context(tc.tile_pool(name="d", bufs=2))
        # one-hot PSUM lives ~2 chunks (emission skew): 2 tags x 3 bufs;
        # diag PSUM is consumed same-iteration: 2 tags x 1 buf -> 8 banks
        ppool2 = ctx.enter_context(
            tc.tile_pool(name="p2", bufs=3, space=bass.MemorySpace.PSUM)
        )
        ppool1 = ctx.enter_context(
            tc.tile_pool(name="p1", bufs=1, space=bass.MemorySpace.PSUM)
        )

        idx_t = const_pool.tile([128, 2 * N // 16], i16, tag="idx1", name="idx1")
        nc.sync.dma_start(idx_t[:], idx1[:])

        # dummy gather: forces the Q7 gather-ucode IRAM load (~6 us) to
        # happen while the input DMAs are still in flight
        idxd = const_pool.tile([128, 1], i16, tag="idxd", name="idxd")
        nc.vector.memset(idxd[:], 0)
        gd = const_pool.tile([128, 1, 2 * C], f8, tag="gd", name="gd")
        nc.gpsimd.dma_gather(gd[:], fap1, idxd[:], 16, 16, 2 * C, elem_step=C)

        # small weight tensors first: the HWDGE ring drains FIFO, so these
        # must not queue behind the 3.1 MB map load
        w2_t = const_pool.tile([128, sums, 128], f8, tag="w2m", name="w2m")
        nc.sync.dma_start(w2_t[:], w2m[:])
        wd_t = const_pool.tile([128, NCHUNK * 4, 128], f8, tag="wd1", name="wd1")
        nc.sync.dma_start(wd_t[:], wd1[:])
        map_t = []
        for a in range(4):
            t = const_pool.tile([128, 8, C], f8, tag=f"map{a}", name=f"map{a}")
            nc.sync.dma_start(t[:], featI2[:, a * 8 * C:(a + 1) * 8 * C])
            map_t.append(t)

        res = []
        for nm in ("n1", "n2", "dot"):
            res.append(const_pool.tile([128, 2 * NCHUNK], f32,
                                       tag=f"res_{nm}", name=f"res_{nm}"))

        gt = []
        for q in range(NG):
            g = gpool.tile([128, 4, 2 * C], f8, tag="g1", name="g1")
            nc.gpsimd.dma_gather(
                g[:], fap1, idx_t[:, q * 32:(q + 1) * 32],
                512, 512, 2 * C, elem_step=C,
            )
            gt.append(g)

        # emission skew: one-hot for chunk c runs ~2 chunks ahead of the
        # gather-dependent diag path, so diag's gather waits never block
        # ready one-hot matmuls on the in-order PE queue
        SKEW = 2
        ps2_all = {}
        for it in range(NCHUNK + SKEW):
            if it < NCHUNK:
                cch = it
                t_lo, span = windows[cch]
                ps2 = [ppool2.tile([128, CH], f32, tag=f"ps2{h}",
                                   name=f"ps2{h}") for h in range(2)]
                ps2_all[cch] = ps2
                for i in range(span):
                    t = t_lo + i
                    lhsT = w2_t[:, int(offs[cch]) + i, :]
                    for h in range(2):
                        rhs = map_t[t // 8][:, t % 8, h * CH:(h + 1) * CH]
                        nc.tensor.matmul(
                            ps2[h][:], lhsT, rhs,
                            start=(i == 0), stop=(i == span - 1),
                        )
            if it < SKEW:
                continue
            cch = it - SKEW
            q, c_loc = cch // 2, cch % 2
            ps2 = ps2_all.pop(cch)
            ps1 = [ppool1.tile([128, CH], f32, tag=f"ps1{h}", name=f"ps1{h}")
                   for h in range(2)]
            for nb in range(4):
                j, x = nb // 2, nb % 2
                lhsT = wd_t[:, cch * 4 + nb, :]
                for h in range(2):
                    rhs = gt[q][:, 2 * c_loc + j,
                                x * C + h * CH: x * C + (h + 1) * CH]
                    nc.tensor.matmul(
                        ps1[h][:], lhsT, rhs,
                        start=(nb == 0), stop=(nb == 3),
                    )
            for h in range(2):
                col = 2 * cch + h
                da = dpool.tile([128, CH], f32, tag=f"da{h}", name=f"da{h}")
                db = dpool.tile([128, CH], f32, tag=f"db{h}", name=f"db{h}")
                dd = dpool.tile([128, CH], f32, tag=f"dd{h}", name=f"dd{h}")
                f1s = dpool.tile([128, CH], f32, tag=f"f1s{h}", name=f"f1s{h}")
                nc.scalar.activation(
                    da[:], ps1[h][:], mybir.ActivationFunctionType.Square,
                    accum_out=res[0][:, col:col + 1],
                )
                nc.scalar.activation(
                    db[:], ps2[h][:], mybir.ActivationFunctionType.Square,
                    accum_out=res[1][:, col:col + 1],
                )
                # DVE may read only one PSUM operand: stage f1 in SBUF
                nc.vector.tensor_copy(f1s[:], ps1[h][:])
                nc.vector.scalar_tensor_tensor(
                    dd[:], ps2[h][:], 1.0, f1s[:],
                    MULT, MULT, accum_out=res[2][:, col:col + 1],
                )

        nc.sync.dma_start(out_n1[:], res[0][:])
        nc.sync.dma_start(out_n2[:], res[1][:])
        nc.sync.dma_start(out_dot[:], res[2][:])

    nc.compile()
    return nc


def get_nc(windows):
    key = tuple(windows)
    if key not in _CACHE:
        _CACHE[key] = _build_nc(windows)
    return _CACHE[key]


def _corner_data(kp_b):
    x = np.asarray(kp_b[:, 0], np.float32)
    y = np.asarray(kp_b[:, 1], np.float32)
    x0 = np.minimum(np.floor(x), np.float32(W - 2)).astype(np.float32)
    y0 = np.minimum(np.floor(y), np.float32(H - 2)).astype(np.float32)
    wx = (x - x0).astype(np.float32)
    wy = (y - y0).astype(np.float32)
    pix0 = y0.astype(np.int32) * W + x0.astype(np.int32)
    w = np.stack(
        [(1 - wx) * (1 - wy), wx * (1 - wy), (1 - wx) * wy, wx * wy], 0
    ).astype(np.float32)
    return pix0, w


def _featT_f8(feat_b):
    featT = np.ascontiguousarray(np.asarray(feat_b, np.float32).reshape(C, HW_).T)
    return np.clip(featT, -240.0, 240.0).astype(F8)


def _make_idx_layout(pix0s):
    """sorted pair-start rows [N] -> [128, 2N/16] int16 gather-index layout.
    Call q (512 idxs) covers chunks [2q, 2q+2): position i = q*512+s*128+p,
    chunk c = 2q + s//2, y-half j = s%2, value = pix0s[c*128+p] + 64*j."""
    i = np.arange(2 * N)
    q, r = i // 512, i % 512
    s, p = r // 128, r % 128
    cc, j = 2 * q + s // 2, s % 2
    vals = pix0s[cc * 128 + p] + W * j
    lay = vals.reshape(-1, 16).T
    return np.tile(lay, (8, 1)).astype(np.int16)


def _make_wd1(w1s):
    wd = np.zeros((128, NCHUNK * 4, 128), np.float32)
    r = np.arange(128)
    for ch in range(NCHUNK):
        for nb in range(4):
            wd[r, ch * 4 + nb, r] = w1s[nb, ch * 128:(ch + 1) * 128]
    return wd.astype(F8)


def _chunk_ranges(pix2s):
    """per-chunk (t_lo, t_hi) of map tiles touched by img2 corners"""
    out = []
    for cc in range(NCHUNK):
        pp = pix2s[cc * 128:(cc + 1) * 128]
        t_lo = int(pp.min()) // 128
        t_hi = (int(pp.max()) + W + 1) // 128
        out.append((t_lo, t_hi))
    return out


def _make_w2m(pix2s, w2s, windows):
    sums = sum(s for _, s in windows)
    w2m = np.zeros((128, sums, 128), np.float32)
    off = 0
    for cc in range(NCHUNK):
        t_lo, span = windows[cc]
        k = np.arange(128)
        for nb in range(4):
            pix = pix2s[cc * 128:(cc + 1) * 128] + (nb % 2) + W * (nb // 2)
            blk = pix // 128 - t_lo
            row = pix % 128
            np.add.at(w2m, (row, off + blk, k), w2s[nb, cc * 128:(cc + 1) * 128])
        off += span
    return w2m.astype(F8)


def _host_inputs(feat1, feat2, kp1, kp2):
    """returns (in_maps, perms, windows)"""
    pre = []
    ranges = []
    for b in range(B):
        pix1, w1 = _corner_data(kp1[b])
        pix2, w2 = _corner_data(kp2[b])
        perm = np.argsort(pix2, kind="stable")
        pre.append((pix1[perm], w1[:, perm], pix2[perm], w2[:, perm], perm))
        ranges.append(_chunk_ranges(pre[-1][2]))
    # shared windows across cores (SPMD: one program for all 8)
    windows = []
    for cc in range(NCHUNK):
        t_lo = min(r[cc][0] for r in ranges)
        t_hi = max(r[cc][1] for r in ranges)
        windows.append((t_lo, t_hi - t_lo + 1))
    windows = tuple(windows)

    in_maps = []
    perms = []
    for b in range(B):
        pix1s, w1s, pix2s, w2s, perm = pre[b]
        in_maps.append({
            "featT1": _featT_f8(feat1[b]),
            "featI2": np.ascontiguousarray(
                _featT_f8(feat2[b]).reshape(NTILE, 128, C)
                .transpose(1, 0, 2).reshape(128, NTILE * C)),
            "idx1": _make_idx_layout(pix1s),
            "wd1": _make_wd1(w1s),
            "w2m": _make_w2m(pix2s, w2s, windows),
        })
        perms.append(perm)
    return in_maps, perms, windows


def kernel(feat1, feat2, kp1, kp2, kp1_mask, kp2_mask):
    from concourse.bass_utils import run_bass_kernel_spmd

    feat1 = np.asarray(feat1, np.float32)
    feat2 = np.asarray(feat2, np.float32)
    kp1 = np.asarray(kp1, np.float32)
    kp2 = np.asarray(kp2, np.float32)
    kp1_mask = np.asarray(kp1_mask)
    kp2_mask = np.asarray(kp2_mask)

    in_maps, perms, windows = _host_inputs(feat1, feat2, kp1, kp2)
    nc = get_nc(windows)
    results = run_bass_kernel_spmd(nc, in_maps, list(range(B))).results

    sum_l2 = 0.0
    sum_valid = 0.0
    for b in range(B):
        r = results[b]

        def unpack(a):
            return (a.reshape(128, NCHUNK, 2).sum(-1)
                    .T.reshape(-1).astype(np.float64))
        n1sq = unpack(r["out_n1"])
        n2sq = unpack(r["out_n2"])
        dot = unpack(r["out_dot"])
        m1 = np.maximum(np.sqrt(n1sq), 1e-12)
        m2 = np.maximum(np.sqrt(n2sq), 1e-12)
        l2 = n1sq / (m1 * m1) + n2sq / (m2 * m2) - 2.0 * dot / (m1 * m2)
        valid = (kp1_mask[b] & kp2_mask[b]).astype(np.float64)[perms[b]]
        sum_l2 += float((l2 * valid).sum())
        sum_valid += float(valid.sum())

    loss = 0.0 if sum_valid == 0 else sum_l2 / max(sum_valid, 1.0)
    return np.float32(loss)


# revision 19
# speedup vs baseline: 1.0339x; 1.0339x over previous
"""Trainium2 Bass kernel for KeypointAlignmentLossL2 — split-path version.

Data-parallel over batch (1 NeuronCore per batch element). The dot product
needs f1[kp] and f2[kp] on the same partition, so both images share one
keypoint ordering: kps are sorted by their img2 pixel-row index.

  - img2: full pixel-major feature map bulk-loaded to SBUF (fp8, 3.1 MB,
    HWDGE — zero Q7 time). Sorted chunks of 128 kps touch only a narrow
    window of 128-pixel-row tiles, so gather+bilinear-lerp fuses into a few
    accumulating one-hot matmuls per chunk (sparse weight mats built on
    host, ~0.9 MB).
  - img1: dma_gather of 2-row pair spans per kp (sorted order; gather does
    not care about locality). Only image 1 pays Q7 descriptor-gen time
    (~17 us). A dummy 16-idx gather issued first forces the ~6 us Q7
    gather-ucode IRAM load to overlap the input DMAs.
  - lerp img1: diagonal-matrix matmuls; squares on ScalarE (PSUM read,
    accum_out); dot via scalar_tensor_tensor accum_out fusion on VectorE
    (tensor_tensor_reduce mis-executes on HW).
  Host finishes: masked mean of l2 distances across cores.
"""
import numpy as np
import ml_dtypes

B, C, H, W, N = 8, 768, 64, 64, 1024
HW_ = H * W
NCHUNK = N // 128   # 8 chunks of 128 keypoints
NTILE = HW_ // 128  # 32 map tiles of 128 pixel-rows
CH = C // 2         # 384 = one PSUM bank of f32
NG = 4              # img1 gather calls (256 idxs = 2 chunks each)

F8 = ml_dtypes.float8_e4m3

_CACHE = {}


def _build_nc(windows):
    """windows: tuple of (t_lo, span) per chunk for the img2 one-hot path."""
    from contextlib import ExitStack
    import concourse.bass as bass
    import concourse.tile as tile
    import concourse.mybir as mybir
    from concourse import bacc

    f32 = mybir.dt.float32
    f8 = mybir.dt.float8e4
    i16 = mybir.dt.int16

    sums = sum(s for _, s in windows)
    offs = np.cumsum([0] + [s for _, s in windows])[:-1]

    # the Tile scheduler orders instructions with the CoreSim cost model,
    # whose SWDGE constant (0.34 ns/desc) is ~25x faster than measured HW
    # gather desc-gen (~7.3 ns/idx); with the default it schedules
    # gather-dependent matmuls first and stalls the in-order PE queue
    from concourse import hw_specs
    old_swdge = hw_specs.TRN2Spec.SWDGE_NS_PER_DESCRIPTOR
    hw_specs.TRN2Spec.SWDGE_NS_PER_DESCRIPTOR = 7.3

    nc = bacc.Bacc("TRN2", target_bir_lowering=False, debug=False, num_devices=8)

    # img1 map stored row-duplicated: dup[r] = [featT[r], featT[r+64]], so
    # ONE gather index fetches a kp's 4 bilinear corners as 3072
    # contiguous bytes (2 dup-rows) -> 1024 gather idxs instead of 4096
    featT1 = nc.dram_tensor("featT1", [HW_, 2 * C], f8, kind="ExternalInput")
    featI2 = nc.dram_tensor("featI2", [128, NTILE * C], f8, kind="ExternalInput")
    idx1 = nc.dram_tensor("idx1", [128, N // 16], i16, kind="ExternalInput")
    wd1 = nc.dram_tensor("wd1", [128, NCHUNK * 4, 128], f8, kind="ExternalInput")
    w2m = nc.dram_tensor("w2m", [128, sums, 128], f8, kind="ExternalInput")
    out_n1 = nc.dram_tensor("out_n1", [128, 2 * NCHUNK], f32, kind="ExternalOutput")
    out_n2 = nc.dram_tensor("out_n2", [128, 2 * NCHUNK], f32, kind="ExternalOutput")
    out_dot = nc.dram_tensor("out_dot", [128, 2 * NCHUNK], f32, kind="ExternalOutput")

    MULT = mybir.AluOpType.mult

    fap1 = featT1[:]
    fap1.ap[0] = [2 * C, HW_ - 1]
    fap1.ap[1] = [1, 4 * C]

    with tile.TileContext(nc) as tc, ExitStack() as ctx:
        const_pool = ctx.enter_context(tc.tile_pool(name="const", bufs=1))
        gpool = ctx.enter_context(tc.tile_pool(name="g", bufs=3))
        dpool = ctx.enter_context(tc.tile_pool(name="d", bufs=2))
        # one-hot PSUM lives ~2 chunks (emission skew): 2 tags x 3 bufs;
        # diag PSUM is consumed same-iteration: 2 tags x 1 buf -> 8 banks
        ppool2 = ctx.enter_context(
            tc.tile_pool(name="p2", bufs=3, space=bass.MemorySpace.PSUM)
        )
        ppool1 = ctx.enter_context(
            tc.tile_pool(name="p1", bufs=1, space=bass.MemorySpace.PSUM)
        )

        idx_t = const_pool.tile([128, N // 16], i16, tag="idx1", name="idx1")
        nc.sync.dma_start(idx_t[:], idx1[:])

        # dummy gather: forces the Q7 gather-ucode IRAM load (~6 us) to
        # happen while the input DMAs are still in flight
        idxd = const_pool.tile([128, 1], i16, tag="idxd", name="idxd")
        nc.vector.memset(idxd[:], 0)
        gd = const_pool.tile([128, 1, 4 * C], f8, tag="gd", name="gd")
        nc.gpsimd.dma_gather(gd[:], fap1, idxd[:], 16, 16, 4 * C, elem_step=2 * C)

        # small weight tensors first: the HWDGE ring drains FIFO, so these
        # must not queue behind the 3.1 MB map load
        w2_t = const_pool.tile([128, sums, 128], f8, tag="w2m", name="w2m")
        nc.sync.dma_start(w2_t[:], w2m[:])
        wd_t = const_pool.tile([128, NCHUNK * 4, 128], f8, tag="wd1", name="wd1")
        nc.sync.dma_start(wd_t[:], wd1[:])
        map_t = []
        for a in range(4):
            t = const_pool.tile([128, 8, C], f8, tag=f"map{a}", name=f"map{a}")
            nc.sync.dma_start(t[:], featI2[:, a * 8 * C:(a + 1) * 8 * C])
            map_t.append(t)

        res = []
        for nm in ("n1", "n2", "dot"):
            res.append(const_pool.tile([128, 2 * NCHUNK], f32,
                                       tag=f"res_{nm}", name=f"res_{nm}"))

        gt = []
        for q in range(NG):
            g = gpool.tile([128, 2, 4 * C], f8, tag="g1", name="g1")
            nc.gpsimd.dma_gather(
                g[:], fap1, idx_t[:, q * 16:(q + 1) * 16],
                256, 256, 4 * C, elem_step=2 * C,
            )
            gt.append(g)

        # emission skew: one-hot for chunk c runs ~2 chunks ahead of the
        # gather-dependent diag path, so diag's gather waits never block
        # ready one-hot matmuls on the in-order PE queue
        SKEW = 2
        ps2_all = {}
        for it in range(NCHUNK + SKEW):
            if it < NCHUNK:
                cch = it
                t_lo, span = windows[cch]
                ps2 = [ppool2.tile([128, CH], f32, tag=f"ps2{h}",
                                   name=f"ps2{h}") for h in range(2)]
                ps2_all[cch] = ps2
                for i in range(span):
                    t = t_lo + i
                    lhsT = w2_t[:, int(offs[cch]) + i, :]
                    for h in range(2):
                        rhs = map_t[t // 8][:, t % 8, h * CH:(h + 1) * CH]
                        nc.tensor.matmul(
                            ps2[h][:], lhsT, rhs,
                            start=(i == 0), stop=(i == span - 1),
                        )
            if it < SKEW:
                continue
            cch = it - SKEW
            q, c_loc = cch // 2, cch % 2
            ps2 = ps2_all.pop(cch)
            ps1 = [ppool1.tile([128, CH], f32, tag=f"ps1{h}", name=f"ps1{h}")
                   for h in range(2)]
            for nb in range(4):
                j, x = nb // 2, nb % 2
                # dup-row gather layout: corner (j, x) at offset x*1536+j*768
                off = x * 2 * C + j * C
                lhsT = wd_t[:, cch * 4 + nb, :]
                for h in range(2):
                    rhs = gt[q][:, c_loc, off + h * CH: off + (h + 1) * CH]
                    nc.tensor.matmul(
                        ps1[h][:], lhsT, rhs,
                        start=(nb == 0), stop=(nb == 3),
                    )
            for h in range(2):
                col = 2 * cch + h
                da = dpool.tile([128, CH], f32, tag=f"da{h}", name=f"da{h}")
                db = dpool.tile([128, CH], f32, tag=f"db{h}", name=f"db{h}")
                dd = dpool.tile([128, CH], f32, tag=f"dd{h}", name=f"dd{h}")
                f1s = dpool.tile([128, CH], f32, tag=f"f1s{h}", name=f"f1s{h}")
                nc.scalar.activation(
                    da[:], ps1[h][:], mybir.ActivationFunctionType.Square,
                    accum_out=res[0][:, col:col + 1],
                )
                nc.scalar.activation(
                    db[:], ps2[h][:], mybir.ActivationFunctionType.Square,
                    accum_out=res[1][:, col:col + 1],
                )
                # DVE may read only one PSUM operand: stage f1 in SBUF
                nc.vector.tensor_copy(f1s[:], ps1[h][:])
                nc.vector.scalar_tensor_tensor(
                    dd[:], ps2[h][:], 1.0, f1s[:],
                    MULT, MULT, accum_out=res[2][:, col:col + 1],
                )

        nc.sync.dma_start(out_n1[:], res[0][:])
        nc.sync.dma_start(out_n2[:], res[1][:])
        nc.sync.dma_start(out_dot[:], res[2][:])

    nc.compile()
    hw_specs.TRN2Spec.SWDGE_NS_PER_DESCRIPTOR = old_swdge
    return nc


def get_nc(windows):
    key = tuple(windows)
    if key not in _CACHE:
        _CACHE[key] = _build_nc(windows)
    return _CACHE[key]


def _corner_data(kp_b):
    x = np.asarray(kp_b[:, 0], np.float32)
    y = np.asarray(kp_b[:, 1], np.float32)
    x0 = np.minimum(np.floor(x), np.float32(W - 2)).astype(np.float32)
    y0 = np.minimum(np.floor(y), np.float32(H - 2)).astype(np.float32)
    wx = (x - x0).astype(np.float32)
    wy = (y - y0).astype(np.float32)
    pix0 = y0.astype(np.int32) * W + x0.astype(np.int32)
    w = np.stack(
        [(1 - wx) * (1 - wy), wx * (1 - wy), (1 - wx) * wy, wx * wy], 0
    ).astype(np.float32)
    return pix0, w


def _featT_f8(feat_b):
    featT = np.ascontiguousarray(np.asarray(feat_b, np.float32).reshape(C, HW_).T)
    return np.clip(featT, -240.0, 240.0).astype(F8)


def _make_idx_layout(pix0s):
    """sorted corner-start rows [N] -> [128, N/16] int16 gather-index layout.
    Call q (256 idxs) covers chunks [2q, 2q+2): position i = q*256+s*128+p,
    chunk c = 2q + s, value = pix0s[c*128+p] (dup-row index)."""
    i = np.arange(N)
    q, r = i // 256, i % 256
    s, p = r // 128, r % 128
    cc = 2 * q + s
    vals = pix0s[cc * 128 + p]
    lay = vals.reshape(-1, 16).T
    return np.tile(lay, (8, 1)).astype(np.int16)


def _make_wd1(w1s):
    wd = np.zeros((128, NCHUNK * 4, 128), np.float32)
    r = np.arange(128)
    for ch in range(NCHUNK):
        for nb in range(4):
            wd[r, ch * 4 + nb, r] = w1s[nb, ch * 128:(ch + 1) * 128]
    return wd.astype(F8)


def _chunk_ranges(pix2s):
    """per-chunk (t_lo, t_hi) of map tiles touched by img2 corners"""
    out = []
    for cc in range(NCHUNK):
        pp = pix2s[cc * 128:(cc + 1) * 128]
        t_lo = int(pp.min()) // 128
        t_hi = (int(pp.max()) + W + 1) // 128
        out.append((t_lo, t_hi))
    return out


def _make_w2m(pix2s, w2s, windows):
    sums = sum(s for _, s in windows)
    w2m = np.zeros((128, sums, 128), np.float32)
    off = 0
    for cc in range(NCHUNK):
        t_lo, span = windows[cc]
        k = np.arange(128)
        for nb in range(4):
            pix = pix2s[cc * 128:(cc + 1) * 128] + (nb % 2) + W * (nb // 2)
            blk = pix // 128 - t_lo
            row = pix % 128
            np.add.at(w2m, (row, off + blk, k), w2s[nb, cc * 128:(cc + 1) * 128])
        off += span
    return w2m.astype(F8)


def _host_inputs(feat1, feat2, kp1, kp2):
    """returns (in_maps, perms, windows)"""
    pre = []
    ranges = []
    for b in range(B):
        pix1, w1 = _corner_data(kp1[b])
        pix2, w2 = _corner_data(kp2[b])
        perm = np.argsort(pix2, kind="stable")
        pre.append((pix1[perm], w1[:, perm], pix2[perm], w2[:, perm], perm))
        ranges.append(_chunk_ranges(pre[-1][2]))
    # shared windows across cores (SPMD: one program for all 8)
    windows = []
    for cc in range(NCHUNK):
        t_lo = min(r[cc][0] for r in ranges)
        t_hi = max(r[cc][1] for r in ranges)
        windows.append((t_lo, t_hi - t_lo + 1))
    windows = tuple(windows)

    in_maps = []
    perms = []
    for b in range(B):
        pix1s, w1s, pix2s, w2s, perm = pre[b]
        fT1 = _featT_f8(feat1[b])
        dup = np.zeros((HW_, 2 * C), F8)
        dup[:HW_ - W, :C] = fT1[:HW_ - W]
        dup[:HW_ - W, C:] = fT1[W:]
        in_maps.append({
            "featT1": dup,
            "featI2": np.ascontiguousarray(
                _featT_f8(feat2[b]).reshape(NTILE, 128, C)
                .transpose(1, 0, 2).reshape(128, NTILE * C)),
            "idx1": _make_idx_layout(pix1s),
            "wd1": _make_wd1(w1s),
            "w2m": _make_w2m(pix2s, w2s, windows),
        })
        perms.append(perm)
    return in_maps, perms, windows


def kernel(feat1, feat2, kp1, kp2, kp1_mask, kp2_mask):
    from concourse.bass_utils import run_bass_kernel_spmd

    feat1 = np.asarray(feat1, np.float32)
    feat2 = np.asarray(feat2, np.float32)
    kp1 = np.asarray(kp1, np.float32)
    kp2 = np.asarray(kp2, np.float32)
    kp1_mask = np.asarray(kp1_mask)
    kp2_mask = np.asarray(kp2_mask)

    in_maps, perms, windows = _host_inputs(feat1, feat2, kp1, kp2)
    nc = get_nc(windows)
    results = run_bass_kernel_spmd(nc, in_maps, list(range(B))).results

    sum_l2 = 0.0
    sum_valid = 0.0
    for b in range(B):
        r = results[b]

        def unpack(a):
            return (a.reshape(128, NCHUNK, 2).sum(-1)
                    .T.reshape(-1).astype(np.float64))
        n1sq = unpack(r["out_n1"])
        n2sq = unpack(r["out_n2"])
        dot = unpack(r["out_dot"])
        m1 = np.maximum(np.sqrt(n1sq), 1e-12)
        m2 = np.maximum(np.sqrt(n2sq), 1e-12)
        l2 = n1sq / (m1 * m1) + n2sq / (m2 * m2) - 2.0 * dot / (m1 * m2)
        valid = (kp1_mask[b] & kp2_mask[b]).astype(np.float64)[perms[b]]
        sum_l2 += float((l2 * valid).sum())
        sum_valid += float(valid.sum())

    loss = 0.0 if sum_valid == 0 else sum_l2 / max(sum_valid, 1.0)
    return np.float32(loss)


# revision 24
# speedup vs baseline: 1.1966x; 1.1574x over previous
"""Trainium2 Bass kernel for KeypointAlignmentLossL2 — split-path version.

Data-parallel over batch (1 NeuronCore per batch element). The dot product
needs f1[kp] and f2[kp] on the same partition, so both images share one
keypoint ordering: kps are sorted by their img2 pixel-row index.

  - img2: full pixel-major feature map bulk-loaded to SBUF (fp8, 3.1 MB,
    HWDGE — zero Q7 time). Sorted chunks of 128 kps touch only a narrow
    window of 128-pixel-row tiles, so gather+bilinear-lerp fuses into a few
    accumulating one-hot matmuls per chunk (sparse weight mats built on
    host, ~0.9 MB).
  - img1: dma_gather of 2-row pair spans per kp (sorted order; gather does
    not care about locality). Only image 1 pays Q7 descriptor-gen time
    (~17 us). A dummy 16-idx gather issued first forces the ~6 us Q7
    gather-ucode IRAM load to overlap the input DMAs.
  - lerp img1: diagonal-matrix matmuls; squares on ScalarE (PSUM read,
    accum_out); dot via scalar_tensor_tensor accum_out fusion on VectorE
    (tensor_tensor_reduce mis-executes on HW).
  Host finishes: masked mean of l2 distances across cores.
"""
import numpy as np
import ml_dtypes

B, C, H, W, N = 8, 768, 64, 64, 1024
HW_ = H * W
NCHUNK = N // 128   # 8 chunks of 128 keypoints
NTILE = HW_ // 128  # 32 map tiles of 128 pixel-rows
CH = C // 2         # 384 = one PSUM bank of f32
NG = 4              # img1 gather calls (256 idxs = 2 chunks each)

F8 = ml_dtypes.float8_e4m3

_CACHE = {}


def _build_nc(windows):
    """windows: tuple of (t_lo, span) per chunk for the img2 one-hot path."""
    from contextlib import ExitStack
    import concourse.bass as bass
    import concourse.tile as tile
    import concourse.mybir as mybir
    from concourse import bacc

    f32 = mybir.dt.float32
    f8 = mybir.dt.float8e4
    i16 = mybir.dt.int16

    sums = sum(s for _, s in windows)
    offs = np.cumsum([0] + [s for _, s in windows])[:-1]

    # the Tile scheduler orders instructions with the CoreSim cost model,
    # whose SWDGE constant (0.34 ns/desc) is ~25x faster than measured HW
    # gather desc-gen (~7.3 ns/idx); with the default it schedules
    # gather-dependent matmuls first and stalls the in-order PE queue
    from concourse import hw_specs
    old_swdge = hw_specs.TRN2Spec.SWDGE_NS_PER_DESCRIPTOR
    old_fixed = hw_specs.TRN2Spec.SWDGE_FIXED_OVERHEAD_NS
    hw_specs.TRN2Spec.SWDGE_NS_PER_DESCRIPTOR = 7.3
    # bias: model the Q7 ucode IRAM-load / startup latency so the scheduler
    # orders gather-independent (one-hot) matmuls ahead of diag ones
    hw_specs.TRN2Spec.SWDGE_FIXED_OVERHEAD_NS = 5000

    nc = bacc.Bacc("TRN2", target_bir_lowering=False, debug=False, num_devices=8)

    # img1 map stored row-duplicated: dup[r] = [featT[r], featT[r+64]], so
    # ONE gather index fetches a kp's 4 bilinear corners as 3072
    # contiguous bytes (2 dup-rows) -> 1024 gather idxs instead of 4096
    featT1 = nc.dram_tensor("featT1", [HW_, 2 * C], f8, kind="ExternalInput")
    featI2 = nc.dram_tensor("featI2", [128, NTILE * C], f8, kind="ExternalInput")
    idx1 = nc.dram_tensor("idx1", [128, N // 16], i16, kind="ExternalInput")
    wd1 = nc.dram_tensor("wd1", [128, NCHUNK * 4, 128], f8, kind="ExternalInput")
    w2m = nc.dram_tensor("w2m", [128, sums, 128], f8, kind="ExternalInput")
    out_n1 = nc.dram_tensor("out_n1", [128, 2 * NCHUNK], f32, kind="ExternalOutput")
    out_n2 = nc.dram_tensor("out_n2", [128, 2 * NCHUNK], f32, kind="ExternalOutput")
    out_dot = nc.dram_tensor("out_dot", [128, 2 * NCHUNK], f32, kind="ExternalOutput")

    MULT = mybir.AluOpType.mult

    fap1 = featT1[:]
    fap1.ap[0] = [2 * C, HW_ - 1]
    fap1.ap[1] = [1, 4 * C]

    with tile.TileContext(nc) as tc, ExitStack() as ctx:
        const_pool = ctx.enter_context(tc.tile_pool(name="const", bufs=1))
        gpool = ctx.enter_context(tc.tile_pool(name="g", bufs=3))
        dpool = ctx.enter_context(tc.tile_pool(name="d", bufs=2))
        # 2 bufs each (8 banks total): lets chunk c+1 matmuls overlap
        # chunk c's epilogue instead of lockstepping on bank recycling
        ppool2 = ctx.enter_context(
            tc.tile_pool(name="p2", bufs=2, space=bass.MemorySpace.PSUM)
        )
        ppool1 = ctx.enter_context(
            tc.tile_pool(name="p1", bufs=2, space=bass.MemorySpace.PSUM)
        )

        idx_t = const_pool.tile([128, N // 16], i16, tag="idx1", name="idx1")
        nc.sync.dma_start(idx_t[:], idx1[:])

        # dummy gather: forces the Q7 gather-ucode IRAM load (~6 us) to
        # happen while the input DMAs are still in flight
        idxd = const_pool.tile([128, 1], i16, tag="idxd", name="idxd")
        nc.vector.memset(idxd[:], 0)
        gd = const_pool.tile([128, 1, 4 * C], f8, tag="gd", name="gd")
        nc.gpsimd.dma_gather(gd[:], fap1, idxd[:], 16, 16, 4 * C, elem_step=2 * C)

        # per-chunk W tiles interleaved with map sub-loads, in the order the
        # one-hot matmuls consume them (HWDGE ring drains FIFO): chunk 0's
        # weights + first map tiles land ~11 us so the PE starts early
        w2c_t = []
        for cc in range(NCHUNK):
            t_lo, span = windows[cc]
            t = const_pool.tile([128, span, 128], f8, tag=f"w2c{cc}",
                                name=f"w2c{cc}")
            w2c_t.append(t)
        map_t = [const_pool.tile([128, 8, C], f8, tag=f"map{a}", name=f"map{a}")
                 for a in range(4)]
        wd_t = const_pool.tile([128, NCHUNK * 4, 128], f8, tag="wd1", name="wd1")

        def w2dma(cc):
            o, (t_lo, span) = int(offs[cc]), windows[cc]
            nc.sync.dma_start(w2c_t[cc][:], w2m[:, o:o + span, :])

        def mapdma(a):
            nc.sync.dma_start(map_t[a][:], featI2[:, a * 8 * C:(a + 1) * 8 * C])

        w2dma(0)
        mapdma(0)
        w2dma(1)
        w2dma(2)
        w2dma(3)
        mapdma(1)
        w2dma(4)
        w2dma(5)
        mapdma(2)
        w2dma(6)
        w2dma(7)
        nc.sync.dma_start(wd_t[:], wd1[:])
        mapdma(3)

        res = []
        for nm in ("n1", "n2", "dot"):
            res.append(const_pool.tile([128, 2 * NCHUNK], f32,
                                       tag=f"res_{nm}", name=f"res_{nm}"))

        gt = []
        for q in range(NG):
            g = gpool.tile([128, 2, 4 * C], f8, tag="g1", name="g1")
            nc.gpsimd.dma_gather(
                g[:], fap1, idx_t[:, q * 16:(q + 1) * 16],
                256, 256, 4 * C, elem_step=2 * C,
            )
            gt.append(g)

        # emission skew: one-hot for chunk c runs ~2 chunks ahead of the
        # gather-dependent diag path, so diag's gather waits never block
        # ready one-hot matmuls on the in-order PE queue
        SKEW = 2
        ps2_all = {}
        for it in range(NCHUNK + SKEW):
            if it < NCHUNK:
                cch = it
                t_lo, span = windows[cch]
                ps2 = [ppool2.tile([128, CH], f32, tag=f"ps2{h}",
                                   name=f"ps2{h}") for h in range(2)]
                ps2_all[cch] = ps2
                for i in range(span):
                    t = t_lo + i
                    lhsT = w2c_t[cch][:, i, :]
                    for h in range(2):
                        rhs = map_t[t // 8][:, t % 8, h * CH:(h + 1) * CH]
                        nc.tensor.matmul(
                            ps2[h][:], lhsT, rhs,
                            start=(i == 0), stop=(i == span - 1),
                        )
            if it < SKEW:
                continue
            cch = it - SKEW
            q, c_loc = cch // 2, cch % 2
            ps2 = ps2_all.pop(cch)
            ps1 = [ppool1.tile([128, CH], f32, tag=f"ps1{h}", name=f"ps1{h}")
                   for h in range(2)]
            for nb in range(4):
                j, x = nb // 2, nb % 2
                # dup-row gather layout: corner (j, x) at offset x*1536+j*768
                off = x * 2 * C + j * C
                lhsT = wd_t[:, cch * 4 + nb, :]
                for h in range(2):
                    rhs = gt[q][:, c_loc, off + h * CH: off + (h + 1) * CH]
                    nc.tensor.matmul(
                        ps1[h][:], lhsT, rhs,
                        start=(nb == 0), stop=(nb == 3),
                    )
            for h in range(2):
                col = 2 * cch + h
                da = dpool.tile([128, CH], f32, tag=f"da{h}", name=f"da{h}")
                db = dpool.tile([128, CH], f32, tag=f"db{h}", name=f"db{h}")
                dd = dpool.tile([128, CH], f32, tag=f"dd{h}", name=f"dd{h}")
                f1s = dpool.tile([128, CH], f32, tag=f"f1s{h}", name=f"f1s{h}")
                nc.scalar.activation(
                    da[:], ps1[h][:], mybir.ActivationFunctionType.Square,
                    accum_out=res[0][:, col:col + 1],
                )
                nc.scalar.activation(
                    db[:], ps2[h][:], mybir.ActivationFunctionType.Square,
                    accum_out=res[1][:, col:col + 1],
                )
                # DVE may read only one PSUM operand: stage f1 in SBUF
                nc.vector.tensor_copy(f1s[:], ps1[h][:])
                nc.vector.scalar_tensor_tensor(
                    dd[:], ps2[h][:], 1.0, f1s[:],
                    MULT, MULT, accum_out=res[2][:, col:col + 1],
                )

        nc.sync.dma_start(out_n1[:], res[0][:])
        nc.sync.dma_start(out_n2[:], res[1][:])
        nc.sync.dma_start(out_dot[:], res[2][:])

    nc.compile()
    hw_specs.TRN2Spec.SWDGE_NS_PER_DESCRIPTOR = old_swdge
    hw_specs.TRN2Spec.SWDGE_FIXED_OVERHEAD_NS = old_fixed
    return nc


def get_nc(windows):
    key = tuple(windows)
    if key not in _CACHE:
        _CACHE[key] = _build_nc(windows)
    return _CACHE[key]


def _corner_data(kp_b):
    x = np.asarray(kp_b[:, 0], np.float32)
    y = np.asarray(kp_b[:, 1], np.float32)
    x0 = np.minimum(np.floor(x), np.float32(W - 2)).astype(np.float32)
    y0 = np.minimum(np.floor(y), np.float32(H - 2)).astype(np.float32)
    wx = (x - x0).astype(np.float32)
    wy = (y - y0).astype(np.float32)
    pix0 = y0.astype(np.int32) * W + x0.astype(np.int32)
    w = np.stack(
        [(1 - wx) * (1 - wy), wx * (1 - wy), (1 - wx) * wy, wx * wy], 0
    ).astype(np.float32)
    return pix0, w


def _featT_f8(feat_b):
    featT = np.ascontiguousarray(np.asarray(feat_b, np.float32).reshape(C, HW_).T)
    return np.clip(featT, -240.0, 240.0).astype(F8)


def _make_idx_layout(pix0s):
    """sorted corner-start rows [N] -> [128, N/16] int16 gather-index layout.
    Call q (256 idxs) covers chunks [2q, 2q+2): position i = q*256+s*128+p,
    chunk c = 2q + s, value = pix0s[c*128+p] (dup-row index)."""
    i = np.arange(N)
    q, r = i // 256, i % 256
    s, p = r // 128, r % 128
    cc = 2 * q + s
    vals = pix0s[cc * 128 + p]
    lay = vals.reshape(-1, 16).T
    return np.tile(lay, (8, 1)).astype(np.int16)


def _make_wd1(w1s):
    wd = np.zeros((128, NCHUNK * 4, 128), np.float32)
    r = np.arange(128)
    for ch in range(NCHUNK):
        for nb in range(4):
            wd[r, ch * 4 + nb, r] = w1s[nb, ch * 128:(ch + 1) * 128]
    return wd.astype(F8)


def _chunk_ranges(pix2s):
    """per-chunk (t_lo, t_hi) of map tiles touched by img2 corners"""
    out = []
    for cc in range(NCHUNK):
        pp = pix2s[cc * 128:(cc + 1) * 128]
        t_lo = int(pp.min()) // 128
        t_hi = (int(pp.max()) + W + 1) // 128
        out.append((t_lo, t_hi))
    return out


def _make_w2m(pix2s, w2s, windows):
    sums = sum(s for _, s in windows)
    w2m = np.zeros((128, sums, 128), np.float32)
    off = 0
    for cc in range(NCHUNK):
        t_lo, span = windows[cc]
        k = np.arange(128)
        for nb in range(4):
            pix = pix2s[cc * 128:(cc + 1) * 128] + (nb % 2) + W * (nb // 2)
            blk = pix // 128 - t_lo
            row = pix % 128
            np.add.at(w2m, (row, off + blk, k), w2s[nb, cc * 128:(cc + 1) * 128])
        off += span
    return w2m.astype(F8)


def _host_inputs(feat1, feat2, kp1, kp2):
    """returns (in_maps, perms, windows)"""
    pre = []
    ranges = []
    for b in range(B):
        pix1, w1 = _corner_data(kp1[b])
        pix2, w2 = _corner_data(kp2[b])
        perm = np.argsort(pix2, kind="stable")
        pre.append((pix1[perm], w1[:, perm], pix2[perm], w2[:, perm], perm))
        ranges.append(_chunk_ranges(pre[-1][2]))
    # shared windows across cores (SPMD: one program for all 8)
    windows = []
    for cc in range(NCHUNK):
        t_lo = min(r[cc][0] for r in ranges)
        t_hi = max(r[cc][1] for r in ranges)
        windows.append((t_lo, t_hi - t_lo + 1))
    windows = tuple(windows)

    in_maps = []
    perms = []
    for b in range(B):
        pix1s, w1s, pix2s, w2s, perm = pre[b]
        fT1 = _featT_f8(feat1[b])
        dup = np.zeros((HW_, 2 * C), F8)
        dup[:HW_ - W, :C] = fT1[:HW_ - W]
        dup[:HW_ - W, C:] = fT1[W:]
        in_maps.append({
            "featT1": dup,
            "featI2": np.ascontiguousarray(
                _featT_f8(feat2[b]).reshape(NTILE, 128, C)
                .transpose(1, 0, 2).reshape(128, NTILE * C)),
            "idx1": _make_idx_layout(pix1s),
            "wd1": _make_wd1(w1s),
            "w2m": _make_w2m(pix2s, w2s, windows),
        })
        perms.append(perm)
    return in_maps, perms, windows


def kernel(feat1, feat2, kp1, kp2, kp1_mask, kp2_mask):
    from concourse.bass_utils import run_bass_kernel_spmd

    feat1 = np.asarray(feat1, np.float32)
    feat2 = np.asarray(feat2, np.float32)
    kp1 = np.asarray(kp1, np.float32)
    kp2 = np.asarray(kp2, np.float32)
    kp1_mask = np.asarray(kp1_mask)
    kp2_mask = np.asarray(kp2_mask)

    in_maps, perms, windows = _host_inputs(feat1, feat2, kp1, kp2)
    nc = get_nc(windows)
    results = run_bass_kernel_spmd(nc, in_maps, list(range(B))).results

    sum_l2 = 0.0
    sum_valid = 0.0
    for b in range(B):
        r = results[b]

        def unpack(a):
            return (a.reshape(128, NCHUNK, 2).sum(-1)
                    .T.reshape(-1).astype(np.float64))
        n1sq = unpack(r["out_n1"])
        n2sq = unpack(r["out_n2"])
        dot = unpack(r["out_dot"])
        m1 = np.maximum(np.sqrt(n1sq), 1e-12)
        m2 = np.maximum(np.sqrt(n2sq), 1e-12)
        l2 = n1sq / (m1 * m1) + n2sq / (m2 * m2) - 2.0 * dot / (m1 * m2)
        valid = (kp1_mask[b] & kp2_mask[b]).astype(np.float64)[perms[b]]
        sum_l2 += float((l2 * valid).sum())
        sum_valid += float(valid.sum())

    loss = 0.0 if sum_valid == 0 else sum_l2 / max(sum_valid, 1.0)
    return np.float32(loss)
